# revision 41
# baseline (speedup 1.0000x reference)
"""Trainium2 Bass kernel for the GravityODECell problem.

Physics per step (dt = 0.1, 5 steps, 3 bodies in 2D per row):
    vec_i = p_i - p_{i+1 mod 3}
    ss_i  = |vec_i|^2
    w_i   = s * clip(ss_i, 1, 28900)^{-1.5}          (s = 0.1*A; equivalent to
            the reference's double-clip + sqrt + cube formulation)
    Fs_i  = vec_i * w_i
    v    += Fs_{i-1} - Fs_i
    p    += 0.1 * v

Sharding: pure data parallel over 8 NeuronCores (batch split), A replicated.
Per core, rows are tiled as [128 partitions x W rows x 6 comps]; all 5 steps
run on-chip per tile (single HBM round trip).

Schedule/engine split (vs the naive all-on-Vector version):
  - The two Fs_{i-1} accumulation adds and the ss clip run on the otherwise
    idle Pool engine; square/ln/exp run on the Activation engine; the Vector
    engine keeps the remaining packed tensor work and the state updates.
  - Tiles are processed in interleaved pairs so each engine works on tile B
    while tile A waits on another engine (hides cross-engine sem latency),
    and the activation-table pass is only offered the single table holding
    square+ln+exp so one hoisted table load serves the whole kernel.
"""

import functools
import os
import sys

import numpy as np

for _p in ("/opt/trn_rl_repo", "/root/.axon_site/_ro/trn_rl_repo"):
    if os.path.isdir(_p) and _p not in sys.path:
        sys.path.insert(0, _p)

import concourse.bass as bass
import concourse.bacc as bacc
import concourse.mybir as mybir
from concourse.bass_utils import run_bass_kernel_spmd
from concourse.hw_specs import get_activation_tables
from concourse.tile import TileContext

N_CORES = 8
P = 128
W = 512                      # rows per partition per tile
TILE_ROWS = P * W            # 65536
DT_STEP = 0.1                # DT / N_STEPS = 0.5 / 5
N_STEPS = 5

F32 = mybir.dt.float32
ALU = mybir.AluOpType
ACTF = mybir.ActivationFunctionType

ACT_TABLE = "natural_log_exp_and_others"


class _Bacc(bacc.Bacc):
    """Bacc whose activation-table pass only sees the one table containing
    square+ln+exp, so a single hoisted load serves the whole kernel
    (table order/indices are preserved; other sets are offered empty)."""

    def insert_act_table_loads(self):
        has_activation = any(
            isinstance(i, mybir.InstActivation)
            for b in self.main_func.blocks
            for i in b.instructions
        )
        if not has_activation:
            return
        tables = [
            (name, s if name == ACT_TABLE else set())
            for name, s in get_activation_tables(self.m.arch).items()
        ]
        bacc._bass_rust.insert_act_table_loads(self, tables)


@functools.lru_cache(maxsize=None)
def _build(b_core: int, s: float):
    """Build the per-core Bass kernel for b_core rows (multiple of TILE_ROWS).

    s = DT_STEP * A is baked in as a compile-time constant (bias of the Exp
    activation); the build is cached per distinct s value.
    """
    n_tiles = b_core // TILE_ROWS
    neg_s = s < 0.0
    ln_s = float(np.log(abs(s)))
    nc = _Bacc()

    # SBUF-resident [128,1] constant holding ln|s| (bias operand of the Exp
    # activation), registered like bass's own const APs.
    lnb_t = nc.alloc_sbuf_tensor("const-lnb", [P, 1], F32)
    nc.gpsimd.memset(lnb_t.ap(), ln_s)
    nc.const_aps.aps[(F32, ln_s)] = lnb_t.ap()
    nc.all_engine_barrier()

    poss_in = nc.declare_dram_parameter("poss", [b_core, 6], F32, isOutput=False)
    vels_in = nc.declare_dram_parameter("vels", [b_core, 6], F32, isOutput=False)
    poss_out = nc.declare_dram_parameter("poss_out", [b_core, 6], F32, isOutput=True)
    vels_out = nc.declare_dram_parameter("vels_out", [b_core, 6], F32, isOutput=True)

    # [b_core, 6] -> [n_tiles, 128, W*6]; each partition holds W contiguous rows.
    pr_in = poss_in.rearrange("(t p w) c -> t p (w c)", t=n_tiles, p=P, w=W)
    vr_in = vels_in.rearrange("(t p w) c -> t p (w c)", t=n_tiles, p=P, w=W)
    pr_out = poss_out.rearrange("(t p w) c -> t p (w c)", t=n_tiles, p=P, w=W)
    vr_out = vels_out.rearrange("(t p w) c -> t p (w c)", t=n_tiles, p=P, w=W)

    # A > 0: v += Fs_prev; v -= Fs.   A < 0: the force flips sign.
    op_prev, op_cur = (ALU.add, ALU.subtract) if not neg_s else (ALU.subtract, ALU.add)

    with TileContext(nc) as tc:
        with (
            tc.tile_pool(name="state", bufs=2) as spool,
            tc.tile_pool(name="tmp", bufs=2) as tpool,
        ):
            tiles = {}

            def load_tile(t):
                tp = spool.tile([P, W, 6], F32, tag="p")
                tv = spool.tile([P, W, 6], F32, tag="v")
                vecs = tpool.tile([P, W, 6], F32, tag="vecs")
                sq = tpool.tile([P, W, 6], F32, tag="sq")
                ss = tpool.tile([P, W, 3], F32, tag="ss")
                lnu = tpool.tile([P, W, 3], F32, tag="lnu")
                r3 = tpool.tile([P, W, 3], F32, tag="r3")
                fs = tpool.tile([P, W, 6], F32, tag="fs")
                tiles[t] = (tp, tv, vecs, sq, ss, lnu, r3, fs)

                tp_f = tp[:, :, :].rearrange("p w c -> p (w c)")
                tv_f = tv[:, :, :].rearrange("p w c -> p (w c)")
                nc.sync.dma_start(out=tp_f, in_=pr_in[t])
                nc.sync.dma_start(out=tv_f, in_=vr_in[t])

            def step_ops(t):
                tp, tv, vecs, sq, ss, lnu, r3, fs = tiles[t]
                tp_f = tp[:, :, :].rearrange("p w c -> p (w c)")
                tv_f = tv[:, :, :].rearrange("p w c -> p (w c)")
                vecs_f = vecs[:, :, :].rearrange("p w c -> p (w c)")
                sq_f = sq[:, :, :].rearrange("p w c -> p (w c)")
                sq4 = sq[:, :, :].rearrange("p w (i two) -> p w i two", two=2)
                vecs4 = vecs[:, :, :].rearrange("p w (i two) -> p w i two", two=2)
                fs4 = fs[:, :, :].rearrange("p w (i two) -> p w i two", two=2)
                fs_f = fs[:, :, :].rearrange("p w c -> p (w c)")
                ss_f = ss[:, :, :].rearrange("p w c -> p (w c)")
                lnu_f = lnu[:, :, :].rearrange("p w c -> p (w c)")
                r3_f = r3[:, :, :].rearrange("p w c -> p (w c)")
                r3b = r3[:, :, :].unsqueeze(3).broadcast_to((P, W, 3, 2))

                return [
                    # vec_i = p_i - p_{i+1 mod 3}  (components interleaved x,y)
                    lambda: nc.vector.tensor_sub(
                        vecs[:, :, 0:4], tp[:, :, 0:4], tp[:, :, 2:6]
                    ),
                    lambda: nc.vector.tensor_sub(
                        vecs[:, :, 4:6], tp[:, :, 4:6], tp[:, :, 0:2]
                    ),
                    # squared components on ScalarE
                    lambda: nc.scalar.activation(sq_f, vecs_f, ACTF.Square),
                    # ss_i = dx^2 + dy^2
                    lambda: nc.vector.tensor_add(
                        ss[:, :, :], sq4[:, :, :, 0], sq4[:, :, :, 1]
                    ),
                    # u = clip(ss, 1, 170^2)  -- on the idle Pool engine
                    lambda: nc.gpsimd.tensor_scalar(
                        ss_f, ss_f, 1.0, 28900.0, ALU.max, ALU.min
                    ),
                    # w = s * u^{-1.5} = exp(-1.5*ln(u) + ln|s|)
                    lambda: nc.scalar.activation(lnu_f, ss_f, ACTF.Ln),
                    lambda: nc.scalar.activation(
                        r3_f, lnu_f, ACTF.Exp, bias=ln_s, scale=-1.5
                    ),
                    # Fs_i = vec_i * w_i
                    lambda: nc.vector.tensor_mul(
                        fs4[:, :, :, :], vecs4[:, :, :, :], r3b
                    ),
                    # v += Fs_{i-1}: the two wrapped adds ride Pool
                    lambda: nc.gpsimd.tensor_tensor(
                        tv[:, :, 0:2], tv[:, :, 0:2], fs[:, :, 4:6], op_prev
                    ),
                    lambda: nc.gpsimd.tensor_tensor(
                        tv[:, :, 2:6], tv[:, :, 2:6], fs[:, :, 0:4], op_prev
                    ),
                    # v -= Fs_i
                    lambda: nc.vector.tensor_tensor(tv_f, tv_f, fs_f, op_cur),
                    # p += 0.1 * v
                    lambda: nc.vector.scalar_tensor_tensor(
                        tp_f, tv_f, DT_STEP, tp_f, ALU.mult, ALU.add
                    ),
                ]

            def store_tile(t):
                tp, tv, *_ = tiles[t]
                tp_f = tp[:, :, :].rearrange("p w c -> p (w c)")
                tv_f = tv[:, :, :].rearrange("p w c -> p (w c)")
                nc.sync.dma_start(out=pr_out[t], in_=tp_f)
                nc.sync.dma_start(out=vr_out[t], in_=tv_f)

            # interleaved pairs: ops of tiles (2k, 2k+1) alternate so engines
            # work on one tile while the other waits on a different engine
            for base in range(0, n_tiles, 2):
                pair = [t for t in (base, base + 1) if t < n_tiles]
                for t in pair:
                    load_tile(t)
                for _step in range(N_STEPS):
                    for ops in zip(*[step_ops(t) for t in pair]):
                        for op in ops:
                            op()
                for t in pair:
                    store_tile(t)

    nc.finalize()
    return nc


def _numpy_reference(poss, vels, A):
    p = poss.astype(np.float32).copy()
    v = vels.astype(np.float32).copy()
    A = np.float32(A)
    for _ in range(N_STEPS):
        b = p.reshape(-1, 3, 2)
        vecs = b - np.roll(b, -1, axis=1)
        ss = np.clip((vecs**2).sum(-1, keepdims=True), 0.1, 100000.0)
        norms = np.sqrt(ss)
        F = vecs / np.clip(norms, 1.0, 170.0) ** 3
        F = -(A * (F - np.roll(F, 1, axis=1)))
        v = v + np.float32(DT_STEP) * F.reshape(-1, 6)
        p = p + np.float32(DT_STEP) * v
    return p, v


def kernel(poss, vels, A):
    poss = np.ascontiguousarray(poss, dtype=np.float32)
    vels = np.ascontiguousarray(vels, dtype=np.float32)
    a_val = float(np.asarray(A))
    s = DT_STEP * a_val

    b_total = poss.shape[0]
    if s == 0.0 or b_total % (N_CORES * TILE_ROWS) != 0:
        return _numpy_reference(poss, vels, a_val)

    b_core = b_total // N_CORES
    nc = _build(b_core, s)

    in_maps = [
        {
            "poss": poss[i * b_core : (i + 1) * b_core],
            "vels": vels[i * b_core : (i + 1) * b_core],
        }
        for i in range(N_CORES)
    ]
    res = run_bass_kernel_spmd(nc, in_maps, list(range(N_CORES)))
    poss_o = np.concatenate([r["poss_out"] for r in res.results], axis=0)
    vels_o = np.concatenate([r["vels_out"] for r in res.results], axis=0)
    return poss_o, vels_o


# revision 43
# speedup vs baseline: 1.8668x; 1.8668x over previous
"""Trainium2 Bass kernel for the GravityODECell problem.

Physics per step (dt = 0.1, 5 steps, 3 bodies in 2D per row):
    vec_i = p_i - p_{i+1 mod 3}
    ss_i  = |vec_i|^2
    w_i   = s * clip(ss_i, 1, 28900)^{-1.5}          (s = 0.1*A; equivalent to
            the reference's double-clip + sqrt + cube formulation)
    Fs_i  = vec_i * w_i
    v    += Fs_{i-1} - Fs_i
    p    += 0.1 * v

Sharding: pure data parallel over 8 NeuronCores (batch split), A replicated.
Per core, rows are tiled as [128 partitions x W rows x 6 comps]; all 5 steps
run on-chip per tile (single HBM round trip).
"""

import functools
import os
import sys

import numpy as np

for _p in ("/opt/trn_rl_repo", "/root/.axon_site/_ro/trn_rl_repo"):
    if os.path.isdir(_p) and _p not in sys.path:
        sys.path.insert(0, _p)

import concourse.bass as bass
import concourse.bacc as bacc
import concourse.mybir as mybir
from concourse.bass_utils import run_bass_kernel_spmd
from concourse.tile import TileContext

N_CORES = 8
P = 128
W = 512                      # rows per partition per tile
TILE_ROWS = P * W            # 65536
DT_STEP = 0.1                # DT / N_STEPS = 0.5 / 5
N_STEPS = 5

F32 = mybir.dt.float32
ALU = mybir.AluOpType
ACTF = mybir.ActivationFunctionType


@functools.lru_cache(maxsize=None)
def _build(b_core: int, s: float):
    """Build the per-core Bass kernel for b_core rows (multiple of TILE_ROWS).

    s = DT_STEP * A is baked in as a compile-time constant (bias of the Exp
    activation); the build is cached per distinct s value.
    """
    n_tiles = b_core // TILE_ROWS
    neg_s = s < 0.0
    ln_s = float(np.log(abs(s)))
    nc = bacc.Bacc()

    # SBUF-resident [128,1] constant holding ln|s| (bias operand of the Exp
    # activation). Written once before the Tile region, like bass's own
    # const APs.
    lnb_t = nc.alloc_sbuf_tensor("const-lnb", [P, 1], F32)
    nc.gpsimd.memset(lnb_t.ap(), ln_s)
    nc.const_aps.aps[(F32, ln_s)] = lnb_t.ap()
    nc.all_engine_barrier()

    poss_in = nc.declare_dram_parameter("poss", [b_core, 6], F32, isOutput=False)
    vels_in = nc.declare_dram_parameter("vels", [b_core, 6], F32, isOutput=False)
    poss_out = nc.declare_dram_parameter("poss_out", [b_core, 6], F32, isOutput=True)
    vels_out = nc.declare_dram_parameter("vels_out", [b_core, 6], F32, isOutput=True)

    # [b_core, 6] -> [n_tiles, 128, W*6]; each partition holds W contiguous rows.
    pr_in = poss_in.rearrange("(t p w) c -> t p (w c)", t=n_tiles, p=P, w=W)
    vr_in = vels_in.rearrange("(t p w) c -> t p (w c)", t=n_tiles, p=P, w=W)
    pr_out = poss_out.rearrange("(t p w) c -> t p (w c)", t=n_tiles, p=P, w=W)
    vr_out = vels_out.rearrange("(t p w) c -> t p (w c)", t=n_tiles, p=P, w=W)

    # A > 0: v += Fs_prev; v -= Fs.   A < 0: the force flips sign.
    op_prev, op_cur = (ALU.add, ALU.subtract) if not neg_s else (ALU.subtract, ALU.add)

    with TileContext(nc) as tc:
        with (
            tc.tile_pool(name="state", bufs=2) as spool,
            tc.tile_pool(name="tmp", bufs=2) as tpool,
        ):
            tiles = {}

            def load_tile(t):
                tp = spool.tile([P, W, 6], F32, tag="p")
                tv = spool.tile([P, W, 6], F32, tag="v")
                vecs = tpool.tile([P, W, 6], F32, tag="vecs")
                sq = tpool.tile([P, W, 6], F32, tag="sq")
                ss = tpool.tile([P, W, 3], F32, tag="ss")
                lnu = tpool.tile([P, W, 3], F32, tag="lnu")
                r3 = tpool.tile([P, W, 3], F32, tag="r3")
                fs = tpool.tile([P, W, 6], F32, tag="fs")
                tiles[t] = (tp, tv, vecs, sq, ss, lnu, r3, fs)

                tp_f = tp[:, :, :].rearrange("p w c -> p (w c)")
                tv_f = tv[:, :, :].rearrange("p w c -> p (w c)")
                nc.sync.dma_start(out=tp_f, in_=pr_in[t])
                nc.sync.dma_start(out=tv_f, in_=vr_in[t])

            def step_ops(t):
                """Step ops as thunks so a pair of tiles can interleave at the
                instruction level: the Vector engine works on the other tile
                while this one waits on ScalarE's square/ln/exp chain."""
                tp, tv, vecs, sq, ss, lnu, r3, fs = tiles[t]
                tp_f = tp[:, :, :].rearrange("p w c -> p (w c)")
                tv_f = tv[:, :, :].rearrange("p w c -> p (w c)")
                vecs_f = vecs[:, :, :].rearrange("p w c -> p (w c)")
                sq_f = sq[:, :, :].rearrange("p w c -> p (w c)")
                sq4 = sq[:, :, :].rearrange("p w (i two) -> p w i two", two=2)
                vecs4 = vecs[:, :, :].rearrange("p w (i two) -> p w i two", two=2)
                fs4 = fs[:, :, :].rearrange("p w (i two) -> p w i two", two=2)
                fs_f = fs[:, :, :].rearrange("p w c -> p (w c)")
                ss_f = ss[:, :, :].rearrange("p w c -> p (w c)")
                lnu_f = lnu[:, :, :].rearrange("p w c -> p (w c)")
                r3_f = r3[:, :, :].rearrange("p w c -> p (w c)")
                r3b = r3[:, :, :].unsqueeze(3).broadcast_to((P, W, 3, 2))

                return [
                    # vec_i = p_i - p_{i+1 mod 3}  (components interleaved x,y)
                    lambda: nc.vector.tensor_sub(
                        vecs[:, :, 0:4], tp[:, :, 0:4], tp[:, :, 2:6]
                    ),
                    lambda: nc.vector.tensor_sub(
                        vecs[:, :, 4:6], tp[:, :, 4:6], tp[:, :, 0:2]
                    ),
                    # squared components on ScalarE
                    lambda: nc.scalar.activation(sq_f, vecs_f, ACTF.Square),
                    # ss_i = dx^2 + dy^2
                    lambda: nc.vector.tensor_add(
                        ss[:, :, :], sq4[:, :, :, 0], sq4[:, :, :, 1]
                    ),
                    # u = clip(ss, 1, 170^2)
                    lambda: nc.vector.tensor_scalar(
                        ss_f, ss_f, 1.0, 28900.0, ALU.max, ALU.min
                    ),
                    # w = s * u^{-1.5} = exp(-1.5*ln(u) + ln|s|)
                    lambda: nc.scalar.activation(lnu_f, ss_f, ACTF.Ln),
                    lambda: nc.scalar.activation(
                        r3_f, lnu_f, ACTF.Exp, bias=ln_s, scale=-1.5
                    ),
                    # Fs_i = vec_i * w_i
                    lambda: nc.vector.tensor_mul(
                        fs4[:, :, :, :], vecs4[:, :, :, :], r3b
                    ),
                    # v += Fs_{i-1} - Fs_i
                    lambda: nc.vector.tensor_tensor(
                        tv[:, :, 0:2], tv[:, :, 0:2], fs[:, :, 4:6], op_prev
                    ),
                    lambda: nc.vector.tensor_tensor(
                        tv[:, :, 2:6], tv[:, :, 2:6], fs[:, :, 0:4], op_prev
                    ),
                    lambda: nc.vector.tensor_tensor(tv_f, tv_f, fs_f, op_cur),
                    # p += 0.1 * v
                    lambda: nc.vector.scalar_tensor_tensor(
                        tp_f, tv_f, DT_STEP, tp_f, ALU.mult, ALU.add
                    ),
                ]

            def store_tile(t):
                tp, tv, *_ = tiles[t]
                tp_f = tp[:, :, :].rearrange("p w c -> p (w c)")
                tv_f = tv[:, :, :].rearrange("p w c -> p (w c)")
                nc.sync.dma_start(out=pr_out[t], in_=tp_f)
                nc.sync.dma_start(out=vr_out[t], in_=tv_f)

            for base in range(0, n_tiles, 2):
                pair = [t for t in (base, base + 1) if t < n_tiles]
                for t in pair:
                    load_tile(t)
                for _step in range(N_STEPS):
                    for ops in zip(*[step_ops(t) for t in pair]):
                        for op in ops:
                            op()
                for t in pair:
                    store_tile(t)

    nc.finalize()
    return nc


def _numpy_reference(poss, vels, A):
    p = poss.astype(np.float32).copy()
    v = vels.astype(np.float32).copy()
    A = np.float32(A)
    for _ in range(N_STEPS):
        b = p.reshape(-1, 3, 2)
        vecs = b - np.roll(b, -1, axis=1)
        ss = np.clip((vecs**2).sum(-1, keepdims=True), 0.1, 100000.0)
        norms = np.sqrt(ss)
        F = vecs / np.clip(norms, 1.0, 170.0) ** 3
        F = -(A * (F - np.roll(F, 1, axis=1)))
        v = v + np.float32(DT_STEP) * F.reshape(-1, 6)
        p = p + np.float32(DT_STEP) * v
    return p, v


def kernel(poss, vels, A):
    poss = np.ascontiguousarray(poss, dtype=np.float32)
    vels = np.ascontiguousarray(vels, dtype=np.float32)
    a_val = float(np.asarray(A))
    s = DT_STEP * a_val

    b_total = poss.shape[0]
    if s == 0.0 or b_total % (N_CORES * TILE_ROWS) != 0:
        return _numpy_reference(poss, vels, a_val)

    b_core = b_total // N_CORES
    nc = _build(b_core, s)

    in_maps = [
        {
            "poss": poss[i * b_core : (i + 1) * b_core],
            "vels": vels[i * b_core : (i + 1) * b_core],
        }
        for i in range(N_CORES)
    ]
    res = run_bass_kernel_spmd(nc, in_maps, list(range(N_CORES)))
    poss_o = np.concatenate([r["poss_out"] for r in res.results], axis=0)
    vels_o = np.concatenate([r["vels_out"] for r in res.results], axis=0)
    return poss_o, vels_o
